# revision 1
# baseline (speedup 1.0000x reference)
"""GCN message-passing (gather + segment-sum) on 8 TRN2 NeuronCores.

out[v] = sum over edges (u -> v) of features[u]

Strategy (dst-sharded, self-contained per core — no collectives):
  - 8 cores each own a 12544-node dst range (8 x 12544 = 100352 >= 100000).
  - Features live in DRAM as a padded table of 256-byte rows ([*, 64] f32,
    payload in [:, :32]) split into 4 chunks of 25088 rows + one zero row
    each, so each chunk is addressable by int16 dma_gather indices.
  - Per (core, section=src-chunk): edges are scheduled by destination;
    dst nodes are ranked by in-degree (descending).  Rank r maps to
    accumulator slot (partition r%128, group r//128); each group of 128
    ranks shares a run length R_g (cross-core max => one static NEFF).
    A node's message slots are consecutive columns of its partition.
  - dma_gather (GPSIMD SWDGE, 4 queues round-robin, 1024-idx batches)
    fills staging tiles [128, cols, 64]; padding slots gather a zero row.
  - DVE tensor_reduce sums each run level (strided X-reduce) into acc
    tiles [128, 98, 64] (payload [:, :, :32]).
  - dma_scatter_add (batched like the gathers) adds acc rows into
    out[node_id]: the scatter applies the rank->node permutation AND
    merges the 4 sections via the DMA CCE.  Indices are unique within a
    section; sections are serialized against each other.
  - Host concatenates the 8 core outputs and trims to 100000 rows.
"""

import numpy as np

import concourse.bass as bass
import concourse.mybir as mybir
from concourse import bacc
from concourse.bass_utils import run_bass_kernel_spmd

# problem constants (hardcoded per harness contract)
N_NODES = 100000
N_EDGES = 1600000
D = 32

P = 128
N_CORES = 8
NODES_PER_CORE = 12544           # 98 * 128
N_GROUPS = NODES_PER_CORE // P   # 98
N_SEC = 4
CHUNK = 25088                    # nodes per src chunk
TROW = CHUNK + 1                 # +1 zero row per chunk
ZROW = CHUNK                     # local index of the zero row
ELEM = 64                        # table row: 64 f32 = 256 B
BATCH = 1024                     # idxs per SWDGE prep (ring cap ~1024-1536)
BCOLS = BATCH // P               # 8 columns per gather batch
NQ = 4                           # SWDGE queues
BLK_TARGET = 96                  # target columns per staging block
SC_PER_SEC = (NODES_PER_CORE + BATCH - 1) // BATCH  # 13 scatter batches/section
IDXW = NODES_PER_CORE // 16      # 784 wrapped scatter-idx columns per section


def _wrap_idx(stream):
    """[n] int stream -> [128, n//16] int16, replicated across the 8 Q7 cores."""
    n = len(stream)
    w = np.asarray(stream, np.int16).reshape(n // 16, 16).T  # pos i -> (i%16, i//16)
    return np.tile(w, (8, 1))


def _build_schedule(src32, dst32):
    core = dst32 // NODES_PER_CORE
    ldst = dst32 - core * NODES_PER_CORE
    sec = src32 // CHUNK
    lsrc = src32 - sec * CHUNK

    flat = (core * N_SEC + sec) * NODES_PER_CORE + ldst
    cnt = np.bincount(flat, minlength=N_CORES * N_SEC * NODES_PER_CORE)
    cnt = cnt.reshape(N_CORES, N_SEC, NODES_PER_CORE).astype(np.int32)

    order = np.argsort(-cnt, axis=2, kind="stable")       # rank -> node
    scnt = -np.sort(-cnt, axis=2)                         # degree at rank (desc)

    # shared per-section group run length: max over cores at each group head
    R_all = scnt[:, :, 0::P].max(axis=0)                  # [N_SEC, 98]

    # rank of each node per (core, sec)
    rank = np.empty_like(order)
    ar = np.arange(NODES_PER_CORE)
    for c in range(N_CORES):
        for s in range(N_SEC):
            rank[c, s, order[c, s]] = ar

    blocks = []          # [s] -> list of (col0, ncols, levels)
    cols = []            # [s] -> padded column count
    colmap_all = []      # [s][g] -> first column of group g
    for s in range(N_SEC):
        R = R_all[s]
        lv = []
        g = 0
        while g < N_GROUPS and R[g] > 0:
            g1 = g
            while g1 + 1 < N_GROUPS and R[g1 + 1] == R[g]:
                g1 += 1
            lv.append((g, g1 + 1, int(R[g])))
            g = g1 + 1

        blks = []
        colmap = np.zeros(N_GROUPS, np.int64)
        state = {"col": 0, "levels": [], "col0": 0, "cols": 0}

        def close_block():
            if not state["levels"]:
                return
            pad = (-state["cols"]) % BCOLS
            state["cols"] += pad
            blks.append((state["col0"], state["cols"], state["levels"]))
            state["col"] = state["col0"] + state["cols"]
            state["col0"] = state["col"]
            state["cols"] = 0
            state["levels"] = []

        for (g0, g1, R_lv) in lv:
            g = g0
            while g < g1:
                room = BLK_TARGET - state["cols"]
                if R_lv > room and state["cols"] > 0:
                    close_block()
                    continue
                take = min(max(1, room // R_lv), g1 - g)
                lcol = state["cols"]
                state["levels"].append((g, g + take, R_lv, lcol))
                for gg in range(g, g + take):
                    colmap[gg] = state["col0"] + lcol + (gg - g) * R_lv
                state["cols"] += take * R_lv
                g += take
                if state["cols"] >= BLK_TARGET:
                    close_block()
        close_block()
        blocks.append(blks)
        cols.append(state["col"])
        colmap_all.append(colmap)

    total_cols = int(sum(cols))
    sec_colbase = np.cumsum([0] + cols)[:-1].astype(np.int64)

    gidx = []
    sidx = []
    for c in range(N_CORES):
        stream = np.full(P * total_cols, ZROW, np.int64)
        for s in range(N_SEC):
            m = (core == c) & (sec == s)
            r = rank[c, s][ldst[m]]
            v = lsrc[m]
            o = np.argsort(r, kind="stable")
            r = r[o]
            v = v[o]
            starts = np.searchsorted(r, ar)
            k = np.arange(len(r)) - starts[r]
            g = r // P
            p = r % P
            j = colmap_all[s][g] + k                 # column within section
            pos = P * (sec_colbase[s] + j) + p
            stream[pos] = v
        gidx.append(_wrap_idx(stream))
        sid = np.concatenate([order[c, s] for s in range(N_SEC)])
        sidx.append(_wrap_idx(sid))

    return {
        "blocks": blocks,
        "cols": cols,
        "sec_colbase": sec_colbase,
        "total_cols": total_cols,
        "gidx": gidx,
        "sidx": sidx,
    }


def _build_nc(sched, reps=1, skip_reduce=False, skip_scatter=False, dbg_fix=()):
    """reps>1 repeats the whole pipeline (for timing; output is then wrong)."""
    blocks = sched["blocks"]
    sec_colbase = sched["sec_colbase"]
    total_cols = sched["total_cols"]

    blkmax = max(ncols for s in range(N_SEC) for (_, ncols, _) in blocks[s])
    nb_per_rep = sum(len(blocks[s]) for s in range(N_SEC))

    nc = bacc.Bacc("TRN2", target_bir_lowering=False, debug=False,
                   num_devices=N_CORES, num_swdge_queues=NQ)

    feat = nc.dram_tensor("feat", [N_SEC * TROW, ELEM], mybir.dt.float32, kind="ExternalInput")
    gidx = nc.dram_tensor("gidx", [P, 8 * total_cols], mybir.dt.int16, kind="ExternalInput")
    sidx = nc.dram_tensor("sidx", [P, N_SEC * IDXW], mybir.dt.int16, kind="ExternalInput")
    out = nc.dram_tensor("out", [NODES_PER_CORE, ELEM], mybir.dt.float32, kind="ExternalOutput")

    gidx_t = nc.alloc_sbuf_tensor("gidx_t", [P, 8 * total_cols], mybir.dt.int16)
    sidx_t = nc.alloc_sbuf_tensor("sidx_t", [P, N_SEC * IDXW], mybir.dt.int16)
    stage = [nc.alloc_sbuf_tensor(f"stage{i}", [P, blkmax * ELEM], mybir.dt.float32) for i in range(2)]
    acc = [nc.alloc_sbuf_tensor(f"acc{i}", [P, N_GROUPS * ELEM], mybir.dt.float32) for i in range(2)]

    # ---- flat block list over reps: (gs, bi, s, col0, ncols, levels) ----
    blist = []
    for rep in range(reps):
        for s in range(N_SEC):
            for (col0, ncols, levels) in blocks[s]:
                blist.append((rep * N_SEC + s, len(blist), s, col0, ncols, levels))
    # last block index (global) of each global section
    last_bi_of_gs = {}
    first_bi_of_gs = {}
    for (gs, bi, s, col0, ncols, levels) in blist:
        last_bi_of_gs[gs] = bi
        first_bi_of_gs.setdefault(gs, bi)

    # ---- SWDGE entry plan (issue order) ----
    entries = []
    for (gs, bi, s, col0, ncols, levels) in blist:
        for k in range(ncols // BCOLS):
            entries.append(("g", gs, bi, s, int(sec_colbase[s] + col0 + k * BCOLS), k * BCOLS))
        if bi == last_bi_of_gs[gs] and not skip_scatter:
            left = NODES_PER_CORE
            kk = 0
            while left > 0:
                n = min(BATCH, left)
                entries.append(("s", gs, s, kk, n))
                left -= n
                kk += 1

    qnext = [None] * NQ
    gq_cnt = [0] * NQ
    sc_idx = 0
    plan = []
    gcum_of_block = {}
    run = [0] * NQ
    qi = 0
    for e in entries:
        q = qi % NQ
        qi += 1
        plan.append((e, q, qnext[q]))
        if e[0] == "g":
            gq_cnt[q] += 1
            qnext[q] = ("g", q, gq_cnt[q])
            run[q] += 1
            gcum_of_block[e[2]] = tuple(run)
        else:
            sc_idx += 1
            qnext[q] = ("s", sc_idx)
    n_scatters = sc_idx
    qcum = []
    lastc = (0,) * NQ
    for bi in range(len(blist)):
        lastc = gcum_of_block.get(bi, lastc)
        qcum.append(lastc)

    with (
        nc.Block() as block,
        nc.semaphore("ld") as ld,
        nc.semaphore("q0") as q0s,
        nc.semaphore("q1") as q1s,
        nc.semaphore("q2") as q2s,
        nc.semaphore("q3") as q3s,
        nc.semaphore("qp0") as qp0,
        nc.semaphore("qp1") as qp1,
        nc.semaphore("qp2") as qp2,
        nc.semaphore("qp3") as qp3,
        nc.semaphore("red") as red,
        nc.semaphore("sd") as sd,
    ):
        qdma = [q0s, q1s, q2s, q3s]
        qprep = [qp0, qp1, qp2, qp3]

        def emit_wait(g, tok):
            if tok is None:
                return
            if tok[0] == "g":
                g.wait_ge(qdma[tok[1]], 16 * tok[2])
            else:
                g.wait_ge(sd, 16 * tok[1])

        @block.gpsimd
        def _(g: bass.BassGpSimd):
            g.dma_start(out=gidx_t[:], in_=gidx[:]).then_inc(ld, 16)
            g.dma_start(out=sidx_t[:], in_=sidx[:]).then_inc(ld, 16)
            g.wait_ge(ld, 32)
            qprep_cnt = [0] * NQ
            seen_blocks = set()
            for (e, q, wait_tok) in plan:
                if e[0] == "g":
                    (_, gs, bi, s, gcol, lc) = e
                    if bi not in seen_blocks:
                        seen_blocks.add(bi)
                        if bi >= 2 and not skip_reduce:
                            g.wait_ge(red, bi - 1)   # staging buf bi-2 reduced
                    emit_wait(g, wait_tok)
                    if "stage" in dbg_fix:
                        bi_, lc_ = 0, 0
                    else:
                        bi_, lc_ = bi, lc
                    s_ = 0 if "table" in dbg_fix else s
                    gcol_ = 0 if "idx" in dbg_fix else gcol
                    g.dma_gather(
                        out_ap=stage[bi_ % 2].ap().rearrange("p (c e) -> p c e", e=ELEM)[:, lc_:lc_ + BCOLS, :],
                        in_ap=feat[s_ * TROW:(s_ + 1) * TROW, :],
                        idxs_ap=gidx_t[:, 8 * gcol_:8 * (gcol_ + BCOLS)],
                        num_idxs=BATCH,
                        num_idxs_reg=BATCH,
                        elem_size=ELEM,
                        prepare_only=True,
                        sem=qdma[q],
                        queue_num=q,
                    ).then_inc(qprep[q], 1)
                else:
                    (_, gs, s, kk, n) = e
                    if kk == 0 and not skip_reduce:
                        g.wait_ge(red, last_bi_of_gs[gs] + 1)   # acc complete
                        if gs > 0:
                            g.wait_ge(sd, 16 * gs * SC_PER_SEC)  # RMW safety
                    emit_wait(g, wait_tok)
                    g.dma_scatter_add(
                        out_ap=out[:],
                        in_ap=acc[gs % 2].ap().rearrange("p (ge e) -> p ge e", e=ELEM)[:, kk * BCOLS:kk * BCOLS + (n + P - 1) // P, :],
                        idxs_ap=sidx_t[:, s * IDXW + kk * (BATCH // 16): s * IDXW + kk * (BATCH // 16) + (n // 16)],
                        num_idxs=n,
                        num_idxs_reg=n,
                        elem_size=ELEM,
                        prepare_only=True,
                        sem=sd,
                        queue_num=q,
                    ).then_inc(qprep[q], 1)
                qprep_cnt[q] += 1
                g.wait_ge(qprep[q], qprep_cnt[q])
                g.trigger_dma(count=1, queue_num=q)
            if n_scatters:
                g.wait_ge(sd, 16 * n_scatters)
            for q in range(NQ):
                if gq_cnt[q]:
                    g.wait_ge(qdma[q], 16 * gq_cnt[q])

        @block.vector
        def _(v: bass.BassEngine):
            if skip_reduce:
                return
            for (gs, bi, s, col0, ncols, levels) in blist:
                if bi == first_bi_of_gs[gs]:
                    if gs >= 2:
                        v.wait_ge(sd, 16 * (gs - 1) * SC_PER_SEC)
                    v.memset(acc[gs % 2].ap(), 0.0)
                for q in range(NQ):
                    if qcum[bi][q] > 0:
                        v.wait_ge(qdma[q], 16 * qcum[bi][q])
                stage_ap = stage[bi % 2].ap().rearrange("p (c e) -> p c e", e=ELEM)
                acc_ap = acc[gs % 2].ap().rearrange("p (ge e) -> p ge e", e=ELEM)
                last = None
                for (g0, g1, R, lcol) in levels:
                    src = stage_ap[:, lcol:lcol + (g1 - g0) * R, 0:D] \
                        .rearrange("p (gr r) d -> p gr d r", r=R)
                    last = v.tensor_reduce(
                        out=acc_ap[:, g0:g1, 0:D],
                        in_=src,
                        axis=mybir.AxisListType.X,
                        op=mybir.AluOpType.add,
                    )
                last.then_inc(red, 1)

    nc.compile()
    return nc


def _run(nc, in_maps):
    try:
        return run_bass_kernel_spmd(nc, in_maps, list(range(N_CORES)))
    except Exception:
        return run_bass_kernel_spmd(nc, in_maps, list(range(N_CORES)))


def _prep_inputs(features, src, dst):
    features = np.asarray(features, np.float32)
    src32 = np.asarray(src).astype(np.int32)
    dst32 = np.asarray(dst).astype(np.int32)
    sched = _build_schedule(src32, dst32)
    fpad = np.zeros((N_CORES * NODES_PER_CORE, D), np.float32)
    fpad[:N_NODES] = features
    tab = np.zeros((N_SEC * TROW, ELEM), np.float32)
    for s in range(N_SEC):
        tab[s * TROW:s * TROW + CHUNK, :D] = fpad[s * CHUNK:(s + 1) * CHUNK]
    in_maps = [
        {"feat": tab, "gidx": sched["gidx"][c], "sidx": sched["sidx"][c]}
        for c in range(N_CORES)
    ]
    return sched, in_maps


def kernel(features, src, dst):
    sched, in_maps = _prep_inputs(features, src, dst)
    nc = _build_nc(sched)
    res = _run(nc, in_maps)
    out = np.concatenate([res.results[c]["out"][:, :D] for c in range(N_CORES)], axis=0)
    return np.ascontiguousarray(out[:N_NODES])


if __name__ == "__main__":
    rng = np.random.default_rng(0)
    feats = rng.standard_normal((N_NODES, D)).astype(np.float32)
    src = rng.integers(0, N_NODES, N_EDGES).astype(np.int64)
    dst = rng.integers(0, N_NODES, N_EDGES).astype(np.int64)
    got = kernel(feats, src, dst)
    exp = np.zeros((N_NODES, D), np.float32)
    np.add.at(exp, dst, feats[src])
    err = np.linalg.norm(got - exp) / np.linalg.norm(exp)
    print("rel err:", err)



# revision 2
# speedup vs baseline: 5.2858x; 5.2858x over previous
"""GCN message-passing (gather + segment-sum) on 8 TRN2 NeuronCores — v3.

out[v] = sum over edges (u -> v) of features[u]

Architecture (no per-edge DMA descriptors; instruction count minimized —
this platform charges ~20-30us per unrolled instruction):
  - 8 cores each own a 12544-node dst range.  Src nodes are split into 8
    shards of 12544; Q7 group g (partitions 16g..16g+15) holds shard g's
    features transposed in SBUF as [16, 12545, 2] bf16 (feat f of node u at
    partition f//2, elem f%2; col 12544 stays zero and is the pad target).
  - Pass 1: ap_gather (InstAPGather; independent int16 idx stream per
    16-partition group) pulls each group's edge stream as columns into
    staging tiles [128, 6144, 2] bf16, ~5 instructions per sweep.  Per
    group, edges are ordered by the group's own dst-degree rank; runs are
    padded to a run-length profile shared across groups/cores and
    DP-quantized to a handful of levels (trades <15% extra columns for
    ~2x fewer DVE instructions).
  - DVE tensor_reduce sums each run level (strided X-reduce over all 128
    partitions) into acc [128, 12544, 2] bf16 (group-rank order; every
    rank covered, so no memset).
  - Pass 2: ONE ap_gather un-permutes acc into node order (idx =
    rank_g(v)) -> mrg [128, 12544, 2] bf16.
  - Lane fold on DVE: [0:64]+=[64:128], [0:32]+=[32:64]; then the Act
    engine DMA-copies partitions [16:32] into the dead region [64:80]
    so the final [0:16]+=[64:80] respects the 32-partition AP alignment.
  - Act engine DMAs the final [16, 12544, 2] bf16 to DRAM; host converts
    to f32 and reshapes.  Host does only index/metadata prep and layout.
"""

import numpy as np
import ml_dtypes

import concourse.bass as bass
import concourse.mybir as mybir
from concourse import bacc
from concourse.bass_utils import run_bass_kernel_spmd

# problem constants (hardcoded per harness contract)
N_NODES = 100000
N_EDGES = 1600000
D = 32

P = 128
N_CORES = 8
NPC = 12544            # dst nodes per core
N_GROUPS = 8           # Q7 groups == src shards
SHARD = 12544          # src nodes per shard
NE1 = SHARD + 1        # table depth (+ zero col)
ZCOL = SHARD
BLK = 6144             # pass-1 columns per gather block

# cost weights for level quantization (ns)
COL_NS = 28.0          # per extra staged column (Q7 gather)
LVL_NS = 30000.0       # per extra DVE reduce instruction


def _wrap(stream):
    """[n] int array -> [16, n//16] int16 (pos i -> row i%16, col i//16)."""
    n = len(stream)
    return np.asarray(stream, np.int16).reshape(n // 16, 16).T


def _quantize_profile(Rprof):
    """DP-optimal segmentation of the sorted degree profile into few levels.

    Returns a full-coverage [NPC] quantized profile (>=1 everywhere).
    """
    prof = np.maximum(Rprof, 1).astype(np.int64)   # cover zero-degree ranks
    # RLE of the (desc-sorted) profile
    vals = []
    cnts = []
    i = 0
    while i < NPC:
        j = i
        while j + 1 < NPC and prof[j + 1] == prof[i]:
            j += 1
        vals.append(int(prof[i]))
        cnts.append(j - i + 1)
        i = j + 1
    m = len(vals)
    lam = LVL_NS / COL_NS                           # columns per level
    # pad[i][j] = extra columns if entries i..j merged at R=vals[i]
    pref_cnt = np.cumsum([0] + cnts)
    pref_cols = np.cumsum([0] + [v * c for v, c in zip(vals, cnts)])
    INF = float("inf")
    best = [INF] * (m + 1)
    arg = [0] * (m + 1)
    best[0] = 0.0
    for j in range(1, m + 1):
        for i in range(j):
            ncnt = pref_cnt[j] - pref_cnt[i]
            ncols = pref_cols[j] - pref_cols[i]
            pad = vals[i] * ncnt - ncols
            c = best[i] + pad + lam
            if c < best[j]:
                best[j] = c
                arg[j] = i
    # recover segments
    segs = []
    j = m
    while j > 0:
        i = arg[j]
        segs.append((i, j))
        j = i
    segs.reverse()
    q = np.empty(NPC, np.int64)
    for (i, j) in segs:
        q[pref_cnt[i]:pref_cnt[j]] = vals[i]
    return q


def _build_schedule(src32, dst32):
    core = dst32 // NPC
    v = dst32 - core * NPC
    grp = src32 // SHARD
    u = src32 - grp * SHARD

    flat = (core * N_GROUPS + grp) * NPC + v
    cnt = np.bincount(flat, minlength=N_CORES * N_GROUPS * NPC)
    cnt = cnt.reshape(N_CORES, N_GROUPS, NPC).astype(np.int32)

    order = np.argsort(-cnt, axis=2, kind="stable")   # [c,g,rank] -> node
    scnt = -np.sort(-cnt, axis=2)                     # deg at rank (desc)
    rank = np.empty_like(order)
    ar = np.arange(NPC)
    for c in range(N_CORES):
        for g in range(N_GROUPS):
            rank[c, g, order[c, g]] = ar

    Rq = _quantize_profile(scnt.max(axis=(0, 1)))     # [NPC], >=1, few levels

    levels = []                                       # (r0, r1, R, col0)
    colstart = np.zeros(NPC, np.int64)
    col = 0
    r = 0
    while r < NPC:
        R = int(Rq[r])
        r1 = r
        while r1 + 1 < NPC and Rq[r1 + 1] == R:
            r1 += 1
        while r <= r1:
            room = BLK - (col % BLK)
            nfit = room // R
            if nfit == 0:
                col += room
                continue
            nv = min(nfit, r1 - r + 1)
            levels.append((r, r + nv, R, col))
            colstart[r:r + nv] = col + np.arange(nv) * R
            col += nv * R
            r += nv
    C = ((col + BLK - 1) // BLK) * BLK
    nblocks = C // BLK

    blk_levels = [[] for _ in range(nblocks)]
    for (r0, r1, R, col0) in levels:
        blk_levels[col0 // BLK].append((r0, r1, R, col0 % BLK))

    g1 = []
    g2 = []
    for c in range(N_CORES):
        rows1 = []
        rows2 = []
        for g in range(N_GROUPS):
            stream = np.full(C, ZCOL, np.int64)
            m = (core == c) & (grp == g)
            rr = rank[c, g][v[m]]
            uu = u[m]
            o = np.argsort(rr, kind="stable")
            rr = rr[o]
            uu = uu[o]
            starts = np.searchsorted(rr, ar)
            k = np.arange(len(rr)) - starts[rr]
            stream[colstart[rr] + k] = uu
            rows1.append(_wrap(stream))
            rows2.append(_wrap(rank[c, g]))
        g1.append(np.vstack(rows1))
        g2.append(np.vstack(rows2))

    return {
        "C": C,
        "nblocks": nblocks,
        "blk_levels": blk_levels,
        "g1": g1,
        "g2": g2,
    }


def _build_nc(sched, reps=1):
    C = sched["C"]
    nblocks = sched["nblocks"]
    blk_levels = sched["blk_levels"]
    BF = mybir.dt.bfloat16

    nc = bacc.Bacc("TRN2", target_bir_lowering=False, debug=False,
                   num_devices=N_CORES)

    tab_d = nc.dram_tensor("tab", [P, NE1 * 2], BF, kind="ExternalInput")
    g1_d = nc.dram_tensor("g1", [P, C // 16], mybir.dt.int16, kind="ExternalInput")
    g2_d = nc.dram_tensor("g2", [P, NPC // 16], mybir.dt.int16, kind="ExternalInput")
    out_d = nc.dram_tensor("out", [16, NPC * 2], BF, kind="ExternalOutput")

    tab_t = nc.alloc_sbuf_tensor("tab_t", [P, NE1 * 2], BF)
    g1_t = nc.alloc_sbuf_tensor("g1_t", [P, C // 16], mybir.dt.int16)
    g2_t = nc.alloc_sbuf_tensor("g2_t", [P, NPC // 16], mybir.dt.int16)
    stage = [nc.alloc_sbuf_tensor(f"st{i}", [P, BLK * 2], BF) for i in range(2)]
    acc = nc.alloc_sbuf_tensor("acc", [P, NPC * 2], BF)
    mrg = nc.alloc_sbuf_tensor("mrg", [P, NPC * 2], BF)

    tab3 = tab_t.ap().rearrange("p (n d) -> p n d", d=2)
    acc3 = acc.ap().rearrange("p (n d) -> p n d", d=2)
    mrg3 = mrg.ap().rearrange("p (n d) -> p n d", d=2)

    with (
        nc.Block() as block,
        nc.semaphore("ld") as ld,
        nc.semaphore("gat") as gat,
        nc.semaphore("red") as red,
        nc.semaphore("g2s") as g2s,
        nc.semaphore("r2") as r2,
        nc.semaphore("lc") as lc,
        nc.semaphore("r3") as r3,
        nc.semaphore("od") as od,
    ):
        @block.gpsimd
        def _(g: bass.BassGpSimd):
            g.dma_start(out=tab_t[:], in_=tab_d[:]).then_inc(ld, 16)
            g.dma_start(out=g1_t[:], in_=g1_d[:]).then_inc(ld, 16)
            g.dma_start(out=g2_t[:], in_=g2_d[:]).then_inc(ld, 16)
            g.wait_ge(ld, 48)
            for rep in range(reps):
                for b in range(nblocks):
                    gi = rep * nblocks + b
                    if gi >= 2:
                        g.wait_ge(red, gi - 1)
                    g.ap_gather(
                        out_ap=stage[gi % 2].ap().rearrange("p (n d) -> p n d", d=2),
                        in_ap=tab3,
                        idxs_ap=g1_t[:, b * (BLK // 16):(b + 1) * (BLK // 16)],
                        channels=P, num_elems=NE1, d=2, num_idxs=BLK,
                    ).then_inc(gat, 1)
                g.wait_ge(red, (rep + 1) * nblocks)
                if rep >= 1:
                    g.wait_ge(od, 16 * rep)        # mrg free (prev out-DMA done)
                g.ap_gather(
                    out_ap=mrg3,
                    in_ap=acc3,
                    idxs_ap=g2_t[:],
                    channels=P, num_elems=NPC, d=2, num_idxs=NPC,
                ).then_inc(g2s, 1)

        @block.vector
        def _(v: bass.BassEngine):
            with nc.allow_low_precision(reason="bf16 acc; reduce rounds once, tol 2e-2"):
                for rep in range(reps):
                    if rep >= 1:
                        v.wait_ge(g2s, rep)        # acc free (prev pass-2 read done)
                    for b in range(nblocks):
                        gi = rep * nblocks + b
                        v.wait_ge(gat, gi + 1)
                        stage3 = stage[gi % 2].ap().rearrange("p (n d) -> p n d", d=2)
                        last = None
                        for (r0, r1, R, lcol) in blk_levels[b]:
                            src = stage3[:, lcol:lcol + (r1 - r0) * R, :] \
                                .rearrange("p (v r) d -> p v d r", r=R)
                            last = v.tensor_reduce(
                                out=acc3[:, r0:r1, :],
                                in_=src,
                                axis=mybir.AxisListType.X,
                                op=mybir.AluOpType.add,
                            )
                        last.then_inc(red, 1)
                    # lane fold: acc is dead after the pass-2 gather; use its
                    # partitions [0:64) as the realignment scratch so every
                    # tensor+tensor add has equal input base partitions.
                    for i, w in enumerate((64, 32, 16)):
                        v.wait_ge(lc, 16 * (3 * rep + i + 1))
                        inst = v.scalar_tensor_tensor(
                            out=mrg3[0:w], in0=mrg3[0:w], scalar=0.0,
                            in1=acc3[0:w],
                            op0=mybir.AluOpType.bypass, op1=mybir.AluOpType.add,
                        )
                        inst.then_inc(r2, 1)

        @block.scalar
        def _(a: bass.BassEngine):
            for rep in range(reps):
                a.wait_ge(g2s, rep + 1)
                a.dma_start(out=acc[0:64, :], in_=mrg[64:128, :]).then_inc(lc, 16)
                a.wait_ge(r2, 3 * rep + 1)
                a.dma_start(out=acc[0:32, :], in_=mrg[32:64, :]).then_inc(lc, 16)
                a.wait_ge(r2, 3 * rep + 2)
                a.dma_start(out=acc[0:16, :], in_=mrg[16:32, :]).then_inc(lc, 16)
                a.wait_ge(r2, 3 * rep + 3)
                a.dma_start(out=out_d[:], in_=mrg[0:16, :]).then_inc(od, 16)

    nc.compile()
    return nc


def _run(nc, in_maps):
    try:
        return run_bass_kernel_spmd(nc, in_maps, list(range(N_CORES)))
    except Exception:
        return run_bass_kernel_spmd(nc, in_maps, list(range(N_CORES)))


def _prep_inputs(features, src, dst):
    features = np.asarray(features, np.float32)
    src32 = np.asarray(src).astype(np.int32)
    dst32 = np.asarray(dst).astype(np.int32)
    sched = _build_schedule(src32, dst32)

    fpad = np.zeros((N_GROUPS * SHARD, D), np.float32)
    fpad[:N_NODES] = features
    tab = np.zeros((P, NE1, 2), ml_dtypes.bfloat16)
    ft = fpad.reshape(N_GROUPS, SHARD, 16, 2)
    for g in range(N_GROUPS):
        tab[16 * g:16 * (g + 1), :SHARD, :] = ft[g].transpose(1, 0, 2)
    tab = np.ascontiguousarray(tab.reshape(P, NE1 * 2))

    in_maps = [
        {"tab": tab, "g1": sched["g1"][c], "g2": sched["g2"][c]}
        for c in range(N_CORES)
    ]
    return sched, in_maps


def kernel(features, src, dst):
    sched, in_maps = _prep_inputs(features, src, dst)
    nc = _build_nc(sched)
    res = _run(nc, in_maps)
    outs = []
    for c in range(N_CORES):
        o = np.asarray(res.results[c]["out"]).astype(np.float32)  # [16, NPC*2]
        outs.append(o.reshape(16, NPC, 2).transpose(1, 0, 2).reshape(NPC, D))
    out = np.concatenate(outs, axis=0)
    return np.ascontiguousarray(out[:N_NODES]).astype(np.float32)


if __name__ == "__main__":
    rng = np.random.default_rng(0)
    feats = rng.standard_normal((N_NODES, D)).astype(np.float32)
    src = rng.integers(0, N_NODES, N_EDGES).astype(np.int64)
    dst = rng.integers(0, N_NODES, N_EDGES).astype(np.int64)
    got = kernel(feats, src, dst)
    exp = np.zeros((N_NODES, D), np.float32)
    np.add.at(exp, dst, feats[src])
    err = np.linalg.norm(got - exp) / np.linalg.norm(exp)
    print("rel err:", err)


# revision 3
# speedup vs baseline: 5.9970x; 1.1346x over previous
"""GCN message-passing (gather + segment-sum) on 8 TRN2 NeuronCores — v3.

out[v] = sum over edges (u -> v) of features[u]

Architecture (no per-edge DMA descriptors; instruction count minimized —
this platform charges ~20-30us per unrolled instruction):
  - 8 cores each own a 12544-node dst range.  Src nodes are split into 8
    shards of 12544; Q7 group g (partitions 16g..16g+15) holds shard g's
    features transposed in SBUF as [16, 12545, 2] bf16 (feat f of node u at
    partition f//2, elem f%2; col 12544 stays zero and is the pad target).
  - Pass 1: ap_gather (InstAPGather; independent int16 idx stream per
    16-partition group) pulls each group's edge stream as columns into
    staging tiles [128, 6144, 2] bf16, ~5 instructions per sweep.  Per
    group, edges are ordered by the group's own dst-degree rank; runs are
    padded to a run-length profile shared across groups/cores and
    DP-quantized to a handful of levels (trades <15% extra columns for
    ~2x fewer DVE instructions).
  - DVE tensor_reduce sums each run level (strided X-reduce over all 128
    partitions) into acc [128, 12544, 2] bf16 (group-rank order; every
    rank covered, so no memset).
  - Pass 2: ONE ap_gather un-permutes acc into node order (idx =
    rank_g(v)) -> mrg [128, 12544, 2] bf16.
  - Lane fold on DVE: [0:64]+=[64:128], [0:32]+=[32:64]; then the Act
    engine DMA-copies partitions [16:32] into the dead region [64:80]
    so the final [0:16]+=[64:80] respects the 32-partition AP alignment.
  - Act engine DMAs the final [16, 12544, 2] bf16 to DRAM; host converts
    to f32 and reshapes.  Host does only index/metadata prep and layout.
"""

import numpy as np
import ml_dtypes

import concourse.bass as bass
import concourse.mybir as mybir
from concourse import bacc
from concourse.bass_utils import run_bass_kernel_spmd

# problem constants (hardcoded per harness contract)
N_NODES = 100000
N_EDGES = 1600000
D = 32

P = 128
N_CORES = 8
NPC = 12544            # dst nodes per core
N_GROUPS = 8           # Q7 groups == src shards
SHARD = 12544          # src nodes per shard
NE1 = SHARD + 1        # table depth (+ zero col)
ZCOL = SHARD
BLK = 6144             # pass-1 columns per gather block

# cost weights for level quantization (ns)
COL_NS = 28.0          # per extra staged column (Q7 gather)
LVL_NS = 30000.0       # per extra DVE reduce instruction


def _wrap(stream):
    """[n] int array -> [16, n//16] int16 (pos i -> row i%16, col i//16)."""
    n = len(stream)
    return np.asarray(stream, np.int16).reshape(n // 16, 16).T


def _quantize_profile(Rprof):
    """DP-optimal segmentation of the sorted degree profile into few levels.

    Returns a full-coverage [NPC] quantized profile (>=1 everywhere).
    """
    prof = np.maximum(Rprof, 1).astype(np.int64)   # cover zero-degree ranks
    # RLE of the (desc-sorted) profile
    vals = []
    cnts = []
    i = 0
    while i < NPC:
        j = i
        while j + 1 < NPC and prof[j + 1] == prof[i]:
            j += 1
        vals.append(int(prof[i]))
        cnts.append(j - i + 1)
        i = j + 1
    m = len(vals)
    lam = LVL_NS / COL_NS                           # columns per level
    # pad[i][j] = extra columns if entries i..j merged at R=vals[i]
    pref_cnt = np.cumsum([0] + cnts)
    pref_cols = np.cumsum([0] + [v * c for v, c in zip(vals, cnts)])
    INF = float("inf")
    best = [INF] * (m + 1)
    arg = [0] * (m + 1)
    best[0] = 0.0
    for j in range(1, m + 1):
        for i in range(j):
            ncnt = pref_cnt[j] - pref_cnt[i]
            ncols = pref_cols[j] - pref_cols[i]
            pad = vals[i] * ncnt - ncols
            c = best[i] + pad + lam
            if c < best[j]:
                best[j] = c
                arg[j] = i
    # recover segments
    segs = []
    j = m
    while j > 0:
        i = arg[j]
        segs.append((i, j))
        j = i
    segs.reverse()
    q = np.empty(NPC, np.int64)
    for (i, j) in segs:
        q[pref_cnt[i]:pref_cnt[j]] = vals[i]
    return q


def _build_schedule(src32, dst32):
    core = dst32 // NPC
    v = dst32 - core * NPC
    grp = src32 // SHARD
    u = src32 - grp * SHARD

    flat = (core * N_GROUPS + grp) * NPC + v
    cnt = np.bincount(flat, minlength=N_CORES * N_GROUPS * NPC)
    cnt = cnt.reshape(N_CORES, N_GROUPS, NPC).astype(np.int32)

    order = np.argsort(-cnt, axis=2, kind="stable")   # [c,g,rank] -> node
    scnt = -np.sort(-cnt, axis=2)                     # deg at rank (desc)
    rank = np.empty_like(order)
    ar = np.arange(NPC)
    for c in range(N_CORES):
        for g in range(N_GROUPS):
            rank[c, g, order[c, g]] = ar

    Rq = _quantize_profile(scnt.max(axis=(0, 1)))     # [NPC], >=1, few levels

    levels = []                                       # (r0, r1, R, col0)
    colstart = np.zeros(NPC, np.int64)
    col = 0
    r = 0
    while r < NPC:
        R = int(Rq[r])
        r1 = r
        while r1 + 1 < NPC and Rq[r1 + 1] == R:
            r1 += 1
        while r <= r1:
            room = BLK - (col % BLK)
            nfit = room // R
            if nfit == 0:
                col += room
                continue
            nv = min(nfit, r1 - r + 1)
            levels.append((r, r + nv, R, col))
            colstart[r:r + nv] = col + np.arange(nv) * R
            col += nv * R
            r += nv
    C = ((col + BLK - 1) // BLK) * BLK
    nblocks = C // BLK

    blk_levels = [[] for _ in range(nblocks)]
    for (r0, r1, R, col0) in levels:
        blk_levels[col0 // BLK].append((r0, r1, R, col0 % BLK))

    g1 = []
    g2 = []
    for c in range(N_CORES):
        rows1 = []
        rows2 = []
        for g in range(N_GROUPS):
            stream = np.full(C, ZCOL, np.int64)
            m = (core == c) & (grp == g)
            rr = rank[c, g][v[m]]
            uu = u[m]
            o = np.argsort(rr, kind="stable")
            rr = rr[o]
            uu = uu[o]
            starts = np.searchsorted(rr, ar)
            k = np.arange(len(rr)) - starts[rr]
            stream[colstart[rr] + k] = uu
            rows1.append(_wrap(stream))
            rows2.append(_wrap(rank[c, g]))
        g1.append(np.vstack(rows1))
        g2.append(np.vstack(rows2))

    return {
        "C": C,
        "nblocks": nblocks,
        "blk_levels": blk_levels,
        "g1": g1,
        "g2": g2,
    }


def _build_nc(sched, reps=1):
    C = sched["C"]
    nblocks = sched["nblocks"]
    blk_levels = sched["blk_levels"]
    BF = mybir.dt.bfloat16

    nc = bacc.Bacc("TRN2", target_bir_lowering=False, debug=False,
                   num_devices=N_CORES)

    tab_d = nc.dram_tensor("tab", [P, NE1 * 2], BF, kind="ExternalInput")
    g1_d = nc.dram_tensor("g1", [P, C // 16], mybir.dt.int16, kind="ExternalInput")
    g2_d = nc.dram_tensor("g2", [P, NPC // 16], mybir.dt.int16, kind="ExternalInput")
    out_d = nc.dram_tensor("out", [16, NPC * 2], BF, kind="ExternalOutput")

    tab_t = nc.alloc_sbuf_tensor("tab_t", [P, NE1 * 2], BF)
    g1_t = nc.alloc_sbuf_tensor("g1_t", [P, C // 16], mybir.dt.int16)
    g2_t = nc.alloc_sbuf_tensor("g2_t", [P, NPC // 16], mybir.dt.int16)
    stage = [nc.alloc_sbuf_tensor(f"st{i}", [P, BLK * 2], BF) for i in range(2)]
    acc = nc.alloc_sbuf_tensor("acc", [P, NPC * 2], BF)
    mrg = nc.alloc_sbuf_tensor("mrg", [P, NPC * 2], BF)

    tab3 = tab_t.ap().rearrange("p (n d) -> p n d", d=2)
    acc3 = acc.ap().rearrange("p (n d) -> p n d", d=2)
    mrg3 = mrg.ap().rearrange("p (n d) -> p n d", d=2)

    with (
        nc.Block() as block,
        nc.semaphore("ld") as ld,
        nc.semaphore("gat") as gat,
        nc.semaphore("red") as red,
        nc.semaphore("g2s") as g2s,
        nc.semaphore("r2") as r2,
        nc.semaphore("lc") as lc,
        nc.semaphore("r3") as r3,
        nc.semaphore("od") as od,
    ):
        @block.gpsimd
        def _(g: bass.BassGpSimd):
            g.dma_start(out=tab_t[:], in_=tab_d[:]).then_inc(ld, 16)
            g.dma_start(out=g1_t[:], in_=g1_d[:]).then_inc(ld, 16)
            g.dma_start(out=g2_t[:], in_=g2_d[:]).then_inc(ld, 16)
            g.wait_ge(ld, 48)
            Rr = g.alloc_register("q7_red")
            Ro = g.alloc_register("q7_od")
            g.reg_alu(Rr, 0, 0, mybir.AluOpType.add)
            g.reg_alu(Ro, 0, 0, mybir.AluOpType.add)
            with g.Fori(0, reps) as _i:
                for b in range(nblocks):
                    if b >= 2:
                        # target = rep*nblocks + (b-2): Rr tracks it
                        g.wait_ge(red, Rr)
                        if b < nblocks - 1:
                            g.reg_alu(Rr, Rr, 1, mybir.AluOpType.add)
                    g.ap_gather(
                        out_ap=stage[b % 2].ap().rearrange("p (n d) -> p n d", d=2),
                        in_ap=tab3,
                        idxs_ap=g1_t[:, b * (BLK // 16):(b + 1) * (BLK // 16)],
                        channels=P, num_elems=NE1, d=2, num_idxs=BLK,
                    ).then_inc(gat, 1)
                # advance to rep*nblocks + nblocks for the all-reduces wait,
                # which is also the next rep's first (b=2) target
                g.reg_alu(Rr, Rr, 3, mybir.AluOpType.add)
                g.wait_ge(red, Rr)
                g.wait_ge(od, Ro)                  # mrg free (prev out-DMA done)
                g.reg_alu(Ro, Ro, 16, mybir.AluOpType.add)
                g.ap_gather(
                    out_ap=mrg3,
                    in_ap=acc3,
                    idxs_ap=g2_t[:],
                    channels=P, num_elems=NPC, d=2, num_idxs=NPC,
                ).then_inc(g2s, 1)

        @block.vector
        def _(v: bass.BassEngine):
            Tg = v.alloc_register("dve_gat")
            Tp = v.alloc_register("dve_g2s")
            Tl = v.alloc_register("dve_lc")
            v.reg_alu(Tg, 0, 0, mybir.AluOpType.add)
            v.reg_alu(Tp, 0, 0, mybir.AluOpType.add)
            v.reg_alu(Tl, 0, 0, mybir.AluOpType.add)
            with nc.allow_low_precision(reason="bf16 acc; reduce rounds once, tol 2e-2"):
                with v.Fori(0, reps) as _j:
                    v.wait_ge(g2s, Tp)             # acc free (prev pass-2 read done)
                    v.reg_alu(Tp, Tp, 1, mybir.AluOpType.add)
                    for b in range(nblocks):
                        v.reg_alu(Tg, Tg, 1, mybir.AluOpType.add)
                        v.wait_ge(gat, Tg)
                        stage3 = stage[b % 2].ap().rearrange("p (n d) -> p n d", d=2)
                        last = None
                        for (r0, r1, R, lcol) in blk_levels[b]:
                            src = stage3[:, lcol:lcol + (r1 - r0) * R, :] \
                                .rearrange("p (v r) d -> p v d r", r=R)
                            last = v.tensor_reduce(
                                out=acc3[:, r0:r1, :],
                                in_=src,
                                axis=mybir.AxisListType.X,
                                op=mybir.AluOpType.add,
                            )
                        last.then_inc(red, 1)
                    # lane fold: acc is dead after the pass-2 gather; use its
                    # partitions [0:64) as the realignment scratch so every
                    # tensor+tensor add has equal input base partitions.
                    for i, w in enumerate((64, 32, 16)):
                        v.reg_alu(Tl, Tl, 16, mybir.AluOpType.add)
                        v.wait_ge(lc, Tl)
                        inst = v.scalar_tensor_tensor(
                            out=mrg3[0:w], in0=mrg3[0:w], scalar=0.0,
                            in1=acc3[0:w],
                            op0=mybir.AluOpType.bypass, op1=mybir.AluOpType.add,
                        )
                        inst.then_inc(r2, 1)

        @block.scalar
        def _(a: bass.BassEngine):
            Ts = a.alloc_register("act_g2s")
            Tr = a.alloc_register("act_r2")
            a.reg_alu(Ts, 0, 0, mybir.AluOpType.add)
            a.reg_alu(Tr, 0, 0, mybir.AluOpType.add)
            with a.Fori(0, reps) as _k:
                a.reg_alu(Ts, Ts, 1, mybir.AluOpType.add)
                a.wait_ge(g2s, Ts)
                a.dma_start(out=acc[0:64, :], in_=mrg[64:128, :]).then_inc(lc, 16)
                for w_src, w_dst in (((32, 64), (0, 32)), ((16, 32), (0, 16))):
                    a.reg_alu(Tr, Tr, 1, mybir.AluOpType.add)
                    a.wait_ge(r2, Tr)
                    a.dma_start(out=acc[w_dst[0]:w_dst[1], :],
                                in_=mrg[w_src[0]:w_src[1], :]).then_inc(lc, 16)
                a.reg_alu(Tr, Tr, 1, mybir.AluOpType.add)
                a.wait_ge(r2, Tr)
                a.dma_start(out=out_d[:], in_=mrg[0:16, :]).then_inc(od, 16)

    nc.compile()
    return nc


def _run(nc, in_maps):
    try:
        return run_bass_kernel_spmd(nc, in_maps, list(range(N_CORES)))
    except Exception:
        return run_bass_kernel_spmd(nc, in_maps, list(range(N_CORES)))


def _prep_inputs(features, src, dst):
    features = np.asarray(features, np.float32)
    src32 = np.asarray(src).astype(np.int32)
    dst32 = np.asarray(dst).astype(np.int32)
    sched = _build_schedule(src32, dst32)

    fpad = np.zeros((N_GROUPS * SHARD, D), np.float32)
    fpad[:N_NODES] = features
    tab = np.zeros((P, NE1, 2), ml_dtypes.bfloat16)
    ft = fpad.reshape(N_GROUPS, SHARD, 16, 2)
    for g in range(N_GROUPS):
        tab[16 * g:16 * (g + 1), :SHARD, :] = ft[g].transpose(1, 0, 2)
    tab = np.ascontiguousarray(tab.reshape(P, NE1 * 2))

    in_maps = [
        {"tab": tab, "g1": sched["g1"][c], "g2": sched["g2"][c]}
        for c in range(N_CORES)
    ]
    return sched, in_maps


def kernel(features, src, dst):
    sched, in_maps = _prep_inputs(features, src, dst)
    nc = _build_nc(sched)
    res = _run(nc, in_maps)
    outs = []
    for c in range(N_CORES):
        o = np.asarray(res.results[c]["out"]).astype(np.float32)  # [16, NPC*2]
        outs.append(o.reshape(16, NPC, 2).transpose(1, 0, 2).reshape(NPC, D))
    out = np.concatenate(outs, axis=0)
    return np.ascontiguousarray(out[:N_NODES]).astype(np.float32)


if __name__ == "__main__":
    rng = np.random.default_rng(0)
    feats = rng.standard_normal((N_NODES, D)).astype(np.float32)
    src = rng.integers(0, N_NODES, N_EDGES).astype(np.int64)
    dst = rng.integers(0, N_NODES, N_EDGES).astype(np.int64)
    got = kernel(feats, src, dst)
    exp = np.zeros((N_NODES, D), np.float32)
    np.add.at(exp, dst, feats[src])
    err = np.linalg.norm(got - exp) / np.linalg.norm(exp)
    print("rel err:", err)
